# revision 2
# baseline (speedup 1.0000x reference)
"""MoE feed-forward (top-2 of 8 experts) on 8 TRN2 NeuronCores.

Strategy (expert-parallel, per the sharding hint):
  - Host: gate in fp64 (logits -> softmax -> top-2), pack each expert's
    routed tokens into a fixed-capacity buffer, one expert per core.
  - Core e (fused, no DRAM spill):
      Phase A: hT = gelu(W1[e]^T x^T + b1[e]) kept entirely in SBUF,
               produced per (f-tile, token-group) from bf16 GEMMs.
      Phase B: yT = (W2[e]^T hT) * wt, accumulated over all 44 f-tiles
               in PSUM per (d-tile, token-group); tokens ride the moving
               dim in BOTH GEMMs, so cost scales with the exact token
               capacity (no 128-token padding).
  - Host: scatter-add the 8 per-expert yT outputs into the dense result.

All matmuls are bf16 (fp32 runs at 1/4 rate); accumulation is fp32 in
PSUM. All DRAM tensors are host-packed into the exact tiled layouts the
kernel consumes, so every DMA line is large and contiguous.
"""

import os

if os.environ.get("JAX_PLATFORMS") == "cpu":
    # The bass kernel executes through the axon PJRT platform; a cpu-only
    # pin would leave no NeuronCores visible.
    os.environ["JAX_PLATFORMS"] = ""

import numpy as np
import ml_dtypes

P = 128
D = 2048
F = 5632
E = 8
TOP_K = 2
N_CORES = 8
KD = D // P  # 16 k-tiles over D (GEMM1 contraction)
FT = F // P  # 44 f-tiles (GEMM2 contraction)
DT = D // P  # 16 d-tiles (GEMM2 output rows)
FBW = 256  # W1 f-columns streamed per block
NFB = F // FBW  # 22 blocks


def _gate_host(flat, gate_w, gate_b):
    """fp64 gating: returns per-token top-k expert ids and softmax scores."""
    logits = flat.astype(np.float64) @ gate_w.astype(np.float64) + gate_b.astype(
        np.float64
    )
    m = logits.max(axis=-1, keepdims=True)
    e = np.exp(logits - m)
    s = e / e.sum(axis=-1, keepdims=True)
    # stable argsort of -s == lax.top_k tie-breaking (lowest index first)
    order = np.argsort(-s, axis=-1, kind="stable")
    top_i = order[:, :TOP_K]
    return top_i, s


def _token_groups(cap):
    """Split cap tokens into ceil(cap/512) near-equal groups (each a
    multiple of 4, <=512). Keeping every group >=256 cols hides the
    107ns LDWEIGHTS behind the matmul stream."""
    k = -(-cap // 512)
    q = -(-cap // (4 * k)) * 4
    out = []
    t0 = 0
    while t0 < cap:
        t1 = min(t0 + q, cap)
        out.append((t0, t1))
        t0 = t1
    return out


def _build_program(cap):
    import concourse.bass as bass
    import concourse.mybir as mybir
    import concourse.tile as tile

    f32 = mybir.dt.float32
    bf16 = mybir.dt.bfloat16
    groups = _token_groups(cap)

    nc = bass.Bass()
    xT = nc.dram_tensor("xT", [P, KD, cap], bf16, kind="ExternalInput")
    w1 = nc.dram_tensor("w1", [NFB, P, KD, FBW], bf16, kind="ExternalInput")
    w2 = nc.dram_tensor("w2", [DT, P, FT, P], bf16, kind="ExternalInput")
    b1 = nc.dram_tensor("b1", [P, FT], f32, kind="ExternalInput")
    wt = nc.dram_tensor("wt", [P, cap], f32, kind="ExternalInput")
    y = nc.dram_tensor("y", [D, cap], f32, kind="ExternalOutput")

    FL = FBW // P  # f-tiles per W1 block

    with tile.TileContext(nc) as tc:
        with (
            tc.tile_pool(name="const", bufs=1) as constp,
            tc.tile_pool(name="w2pool", bufs=3) as w2pool,
            tc.tile_pool(name="htpool", bufs=1) as htpool,
        ):
            hT = htpool.tile([P, FT, cap], bf16)
            w2_tiles = {}

            # ---- Phase A: hT = gelu(w1.T @ x.T + b1), kept in SBUF ----
            with (
                tc.tile_pool(name="xpool", bufs=1) as xpool,
                tc.tile_pool(name="w1pool", bufs=2) as w1pool,
                tc.tile_pool(name="psA", bufs=4, space="PSUM") as psA,
            ):
                xT_sb = xpool.tile([P, KD, cap], bf16)
                # queue order: first token-group of xT, then W1 block 0 in
                # two half-chunks, then the rest of xT, then the consts
                n_first = groups[0][1]
                nc.sync.dma_start(xT_sb[:, :, 0:n_first], xT[:, :, 0:n_first])
                w1_first = w1pool.tile([P, KD, FBW], bf16, tag="w1sb")
                for c in range(2):
                    nc.sync.dma_start(
                        w1_first[:, :, c * P : (c + 1) * P],
                        w1[0, :, :, c * P : (c + 1) * P],
                    )
                for t0 in range(n_first, cap, 512):
                    t1 = min(t0 + 512, cap)
                    nc.sync.dma_start(xT_sb[:, :, t0:t1], xT[:, :, t0:t1])
                b1_sb = constp.tile([P, FT], f32)
                nc.sync.dma_start(b1_sb[:], b1[:, :])
                wt_sb = constp.tile([P, cap], f32)
                nc.sync.dma_start(wt_sb[:], wt[:, :])

                for fb in range(NFB):
                    if fb == 0:
                        w1_sb = w1_first
                    else:
                        w1_sb = w1pool.tile([P, KD, FBW], bf16, tag="w1sb")
                        nc.sync.dma_start(w1_sb[:], w1[fb])
                    if NFB - 6 <= fb < NFB - 3:
                        # prefetch the first 3 W2 d-tiles late in phase A,
                        # on the ACT HWDGE queue so they never delay the
                        # W1/x loads on the sync queue
                        dtp = fb - (NFB - 6)
                        w2_sb = w2pool.tile([P, FT, P], bf16, tag="w2sb")
                        nc.scalar.dma_start(w2_sb[:], w2[dtp])
                        w2_tiles[dtp] = w2_sb
                    for fl in range(FL):
                        ft = fb * FL + fl
                        for n0, n1 in groups:
                            ps = psA.tile([P, 512], f32, tag="psA")
                            for k in range(KD):
                                nc.tensor.matmul(
                                    ps[:, : n1 - n0],
                                    lhsT=w1_sb[:, k, fl * P : (fl + 1) * P],
                                    rhs=xT_sb[:, k, n0:n1],
                                    start=(k == 0),
                                    stop=(k == KD - 1),
                                )
                            nc.scalar.activation(
                                hT[:, ft, n0:n1],
                                ps[:, : n1 - n0],
                                mybir.ActivationFunctionType.Gelu,
                                bias=b1_sb[:, ft : ft + 1],
                            )

            # ---- Phase B: yT = wt * (w2.T @ hT), d-tile per PSUM group ----
            with (
                tc.tile_pool(name="ypool", bufs=3) as ypool,
                tc.tile_pool(name="psB", bufs=3, space="PSUM") as psB,
            ):
                for dt in range(DT):
                    if dt in w2_tiles:
                        w2_sb = w2_tiles.pop(dt)
                    else:
                        w2_sb = w2pool.tile([P, FT, P], bf16, tag="w2sb")
                        nc.scalar.dma_start(w2_sb[:], w2[dt])
                    for n0, n1 in groups:
                        ps = psB.tile([P, 512], f32, tag="psB")
                        for k in range(FT):
                            nc.tensor.matmul(
                                ps[:, : n1 - n0],
                                lhsT=w2_sb[:, k, :],
                                rhs=hT[:, k, n0:n1],
                                start=(k == 0),
                                stop=(k == FT - 1),
                            )
                        yt = ypool.tile([P, 512], f32, tag="yt")
                        nc.vector.tensor_mul(
                            yt[:, : n1 - n0], ps[:, : n1 - n0], wt_sb[:, n0:n1]
                        )
                        nc.gpsimd.dma_start(
                            y[dt * P : (dt + 1) * P, n0:n1], yt[:, : n1 - n0]
                        )

    _split_multi_waits(nc)
    return nc


def _split_multi_waits(nc):
    """The walrus build in this container rejects >1 sync-wait command per
    instruction; hoist extras onto single-wait NOPs just before it."""
    import bass_rust
    import concourse.mybir as mybir

    ctr = 0
    for blk in nc.m.functions[0].blocks:
        insts = blk.instructions
        i = 0
        while i < len(insts):
            inst = insts[i]
            si = inst.sync_info
            if si is None:
                i += 1
                continue
            waits = list(si.on_wait)
            if len(waits) <= 1:
                i += 1
                continue
            for w in waits[:-1]:
                ctr += 1
                nop = bass_rust.InstNoOp(name=f"waitsplit_{ctr}")
                nop.engine = inst.engine
                nop.sync_info = mybir.SyncInfo(on_wait=[w], on_update=[])
                insts.insert(i, nop)
                i += 1
            inst.sync_info = mybir.SyncInfo(
                on_wait=[waits[-1]], on_update=list(si.on_update)
            )
            i += 1


_CACHE = {}


def _get_program(cap):
    if cap not in _CACHE:
        _CACHE[cap] = _build_program(cap)
    return _CACHE[cap]


_RUNNER_CACHE = {}


def _make_runner(nc, n_cores=N_CORES):
    """Persistent jitted shard_map over the bass NEFF (one jax.jit per
    program, reused across kernel() calls)."""
    import jax
    from jax.sharding import Mesh, PartitionSpec
    from jax.experimental.shard_map import shard_map
    import concourse.mybir as mybir
    from concourse import bass2jax
    from concourse.bass2jax import _bass_exec_p, partition_id_tensor

    bass2jax.install_neuronx_cc_hook()

    partition_name = nc.partition_id_tensor.name if nc.partition_id_tensor else None
    in_names, out_names, out_avals, zero_shapes = [], [], [], []
    for alloc in nc.m.functions[0].allocations:
        if not isinstance(alloc, mybir.MemoryLocationSet):
            continue
        name = alloc.memorylocations[0].name
        if alloc.kind == "ExternalInput":
            if name != partition_name:
                in_names.append(name)
        elif alloc.kind == "ExternalOutput":
            out_names.append(name)
            shape = tuple(alloc.tensor_shape)
            dtype = mybir.dt.np(alloc.dtype)
            out_avals.append(jax.core.ShapedArray(shape, dtype))
            zero_shapes.append((shape, dtype))
    n_params = len(in_names)
    n_outs = len(out_avals)
    in_names.extend(out_names)
    if partition_name is not None:
        in_names.append(partition_name)

    def _body(*args):
        operands = list(args)
        if partition_name is not None:
            operands.append(partition_id_tensor())
        outs = _bass_exec_p.bind(
            *operands,
            out_avals=tuple(out_avals),
            in_names=tuple(in_names),
            out_names=tuple(out_names),
            lowering_input_output_aliases=(),
            sim_require_finite=True,
            sim_require_nnan=True,
            nc=nc,
        )
        return tuple(outs)

    devices = jax.devices()[:n_cores]
    mesh = Mesh(np.asarray(devices), ("core",))
    in_specs = (PartitionSpec("core"),) * (n_params + n_outs)
    out_specs = (PartitionSpec("core"),) * len(out_names)
    donate = tuple(range(n_params, n_params + n_outs))
    sharded = jax.jit(
        shard_map(
            _body, mesh=mesh, in_specs=in_specs, out_specs=out_specs, check_rep=False
        ),
        donate_argnums=donate,
        keep_unused=True,
    )

    def run(in_maps):
        per_core = [
            [np.asarray(m[name]) for name in in_names[:n_params]] for m in in_maps
        ]
        concat_in = [
            np.concatenate([per_core[c][i] for c in range(n_cores)], axis=0)
            for i in range(n_params)
        ]
        concat_zeros = [
            np.zeros((n_cores * s[0], *s[1:]), dt) for s, dt in zero_shapes
        ]
        out_arrs = sharded(*concat_in, *concat_zeros)
        return [
            {
                name: np.asarray(out_arrs[i]).reshape(
                    n_cores, *out_avals[i].shape
                )[c]
                for i, name in enumerate(out_names)
            }
            for c in range(n_cores)
        ]

    return run


def _get_runner(cap):
    if cap not in _RUNNER_CACHE:
        _RUNNER_CACHE[cap] = _make_runner(_get_program(cap))
    return _RUNNER_CACHE[cap]


def prepare(x, gate_w, gate_b, W1, b1, W2, b2):
    """Host routing + per-core input packing. Returns (in_maps, idx, cap,
    top_i, scores, flat_shape)."""
    x = np.asarray(x, np.float32)
    B, S, Dx = x.shape
    assert (Dx, W1.shape[2], gate_b.shape[0]) == (D, F, E)
    T = B * S
    flat = x.reshape(T, D)

    top_i, scores = _gate_host(flat, np.asarray(gate_w), np.asarray(gate_b))

    idx = []
    wts = []
    for e in range(E):
        sel = np.where((top_i == e).any(axis=1))[0]
        idx.append(sel)
        wts.append(scores[sel, e].astype(np.float32))
    max_load = max(len(s) for s in idx)
    cap = max(512, -(-max_load // 8) * 8)

    bf = ml_dtypes.bfloat16
    W1 = np.asarray(W1, np.float32)
    W2 = np.asarray(W2, np.float32)
    b1 = np.asarray(b1, np.float32)

    in_maps = []
    for e in range(E):
        n_e = len(idx[e])
        # xT: [P, KD, cap];  xT[p, k, t] = x[t, k*128+p]
        xTe = np.zeros((P, KD, cap), bf)
        xe = flat[idx[e]].astype(bf)  # [n, D]
        xTe[:, :, :n_e] = xe.T.reshape(KD, P, n_e).transpose(1, 0, 2)
        # w1: [NFB, P, KD, FBW];  w1[fb, p, k, j] = W1[k*128+p, fb*FBW+j]
        w1p = np.ascontiguousarray(
            W1[e].astype(bf).reshape(KD, P, NFB, FBW).transpose(2, 1, 0, 3)
        )
        # w2: [DT, P, FT, P];  w2[dt, p, f, j] = W2[f*128+p, dt*128+j]
        w2p = np.ascontiguousarray(
            W2[e].astype(bf).reshape(FT, P, DT, P).transpose(2, 1, 0, 3)
        )
        # b1: [P, FT];  b1p[p, f] = b1[f*128+p]
        b1p = np.ascontiguousarray(b1[e].reshape(FT, P).T)
        # wt: [P, cap] broadcast along partitions
        wte = np.zeros((cap,), np.float32)
        wte[:n_e] = wts[e]
        wtp = np.ascontiguousarray(np.broadcast_to(wte, (P, cap)))
        in_maps.append({"xT": xTe, "w1": w1p, "w2": w2p, "b1": b1p, "wt": wtp})
    return in_maps, idx, cap, top_i, scores, (B, S, T)


def combine(results, idx, top_i, scores, b2, shape):
    B, S, T = shape
    b2 = np.asarray(b2, np.float32)
    out = np.zeros((T, D), np.float32)
    for e in range(E):
        n_e = len(idx[e])
        out[idx[e]] += results[e]["y"][:, :n_e].T
    if np.any(b2):
        w_dense = np.zeros((T, E), np.float32)
        for k in range(TOP_K):
            w_dense[np.arange(T), top_i[:, k]] += scores[
                np.arange(T), top_i[:, k]
            ].astype(np.float32)
        out += w_dense @ b2
    return out.reshape(B, S, D)


def kernel(x, gate_w, gate_b, W1, b1, W2, b2):
    in_maps, idx, cap, top_i, scores, shape = prepare(
        x, gate_w, gate_b, W1, b1, W2, b2
    )
    results = _get_runner(cap)(in_maps)
    return combine(results, idx, top_i, scores, b2, shape)
